# revision 4
# baseline (speedup 1.0000x reference)
"""Trainium2 Bass kernel for nn_Affinity (gnn_message_passing).

M[(a,b),(c,d)] = sum_{j,i} H2[a,j]H2[c,j] H1[b,i]H1[d,i] W[j,i] + diag(Mp),
W[j,i] = X[j] . lamda . Y[i]  (the reference's quirky Me reindexing).

Key structure: block B_{a,c}[b,d] = sum_i H1[b,i]H1[d,i] w_{ac}[i] is itself
sparse -- off-diagonal entries are (multi-edge-merged) values of w_{ac}, and
its diagonal is H1 @ w_{ac}. Folding the 0/1 selection tables (all host-built,
integer-valued) through the algebra, the whole per-core computation reduces to
~12 small matmuls producing two dense value tables:

  WU[u, s] = merged off-diag value of unique graph-1 pair u in slot s's block
  R[b, s]  = diagonal of slot s's block (+ Mp for diagonal slots)

with slots s = the ~360 unique (a,c) block pairs (72 diagonal + unique graph-2
edges), sharded 45-per-core over 8 cores. The host assembles the final
[5184, 5184] matrix by pure index scatter (no host float arithmetic).
"""
import sys
sys.path.insert(0, '/opt/trn_rl_repo')
import numpy as np

N = 72
E = 288
D = 64
NC = 8
S = 48          # slots per core (9 diag + <=36 edge pairs, padded)
UPAD = 288      # unique graph-1 pairs, padded

# PACKA layout [72, 256] bf16 (l1|l2 contiguous for a single relu)
PA = {"l1": (0, 64, 64), "l2": (64, 64, 64), "f1t": (128, 64, 72),
      "csd": (200, 72, S)}
PA_W = 256
# PACKB1 layout [72, 640] bf16
PB1 = {"f2": (0, 72, 64), "s2u": (64, 72, UPAD), "d2u": (352, 72, UPAD)}
PB1_W = 640
# PACKB2 layout [72, 336] bf16
PB2 = {"s2h": (0, 72, 72), "d2h": (72, 72, 72), "u1t": (144, 64, 72),
       "u2t": (216, 64, 72), "oh": (288, 72, S)}
PB2_W = 336


def _split_waits(nc, limit=1):
    """This walrus build rejects instructions with >limit sem waits; move the
    excess onto same-engine NoOps inserted immediately before."""
    import concourse.mybir as mybir
    for f in nc.m.functions:
        for bb in f.blocks:
            new_insts = []
            for inst in bb.instructions:
                si = inst.sync_info
                waits = list(si.on_wait) if si and si.on_wait else []
                if len(waits) > limit:
                    extra, keep = waits[:-limit], waits[-limit:]
                    for i in range(0, len(extra), limit):
                        nop = mybir.InstNoOp(
                            name=nc.get_next_instruction_name(),
                            engine=inst.engine, ins=[], outs=[],
                            sync_info=mybir.SyncInfo(
                                on_wait=extra[i:i + limit], on_update=[]),
                        )
                        nc.register_instruction(nop)
                        new_insts.append(nop)
                    si.on_wait = keep
                new_insts.append(inst)
            bb.instructions[:] = new_insts


def _build_nc():
    import concourse.bass as bass
    import concourse.mybir as mybir
    import concourse.tile as tile

    F32 = mybir.dt.float32
    BF16 = mybir.dt.bfloat16

    nc = bass.Bass(enable_partition_id=False)
    pka_d = nc.declare_dram_parameter("PACKA", [72, PA_W], BF16, isOutput=False)
    pkb1_d = nc.declare_dram_parameter("PACKB1", [72, PB1_W], BF16,
                                       isOutput=False)
    pkb2_d = nc.declare_dram_parameter("PACKB2", [72, PB2_W], BF16,
                                       isOutput=False)
    out_d = nc.declare_dram_parameter("OUT", [96, 4 * S], F32, isOutput=True)

    with tile.TileContext(nc) as tc:
        with tc.tile_pool(name="cst", bufs=1) as cst, \
             tc.tile_pool(name="ps", bufs=1, space="PSUM") as ps:

            pka = cst.tile([72, PA_W], BF16)
            nc.sync.dma_start(out=pka[:], in_=pka_d[:])
            pkb1 = cst.tile([72, PB1_W], BF16)
            nc.gpsimd.dma_start(out=pkb1[:], in_=pkb1_d[:])
            pkb2 = cst.tile([72, PB2_W], BF16)
            nc.scalar.dma_start(out=pkb2[:], in_=pkb2_d[:])

            def fld(pk, tab, nm):
                o, r, w = tab[nm]
                return pk[0:r, o:o + w]

            f1t = fld(pka, PA, "f1t")
            csd = fld(pka, PA, "csd")
            f2 = fld(pkb1, PB1, "f2")
            s2u = fld(pkb1, PB1, "s2u")
            d2u = fld(pkb1, PB1, "d2u")
            s2h = fld(pkb2, PB2, "s2h")
            d2h = fld(pkb2, PB2, "d2h")
            u1t = fld(pkb2, PB2, "u1t")
            u2t = fld(pkb2, PB2, "u2t")
            oh = fld(pkb2, PB2, "oh")

            # output staging; memset early so the padding rows of the R
            # region hold zeros rather than uninitialized SBUF
            sbo = cst.tile([96, 4 * S], F32)
            nc.vector.memset(sbo[:], 0.0)

            # relu of [lamda1 | lamda2] in one shot
            r12 = cst.tile([D, 2 * D], BF16)
            nc.vector.tensor_relu(out=r12[:], in_=pka[0:D, 0:2 * D])

            # FR = F1 @ [r1 | r2]  [72, 128]
            fr_p = ps.tile([72, 2 * D], F32, tag="fr")
            nc.tensor.matmul(out=fr_p[:], lhsT=f1t, rhs=r12[:],
                             start=True, stop=True)
            frb = cst.tile([72, 2 * D], BF16)
            nc.vector.tensor_copy(out=frb[:], in_=fr_p[:])

            # stage-1 feature/table products (independent of the FR chain)
            # s1 = [FSU(288) | FSR(72)],  s2 = [FDU(288) | FDR(72)]  [64, 360]
            s1_p = ps.tile([D, UPAD + 72], F32, tag="s1")
            nc.tensor.matmul(out=s1_p[:, 0:UPAD], lhsT=f2, rhs=s2u,
                             start=True, stop=True)
            nc.tensor.matmul(out=s1_p[:, UPAD:UPAD + 72], lhsT=f2, rhs=s2h,
                             start=True, stop=True)
            s2_p = ps.tile([D, UPAD + 72], F32, tag="s2")
            nc.tensor.matmul(out=s2_p[:, 0:UPAD], lhsT=f2, rhs=d2u,
                             start=True, stop=True)
            nc.tensor.matmul(out=s2_p[:, UPAD:UPAD + 72], lhsT=f2, rhs=d2h,
                             start=True, stop=True)
            mp_p = ps.tile([72, 72], F32, tag="mp")
            nc.tensor.matmul(out=mp_p[:], lhsT=u1t, rhs=u2t,
                             start=True, stop=True)

            # P = [P1 ; P2] = frb^T @ CSD  [128, S]
            p_p = ps.tile([128, S], F32, tag="p")
            nc.tensor.matmul(out=p_p[:], lhsT=frb[:], rhs=csd,
                             start=True, stop=True)

            # stacked [FSU;FDU | FSR;FDR] in SBUF: rows 0:64 from s1,
            # 64:128 from s2 (disjoint partitions, two engines in parallel)
            sfb = cst.tile([128, UPAD + 72], BF16)
            nc.scalar.copy(out=sfb[0:D, :], in_=s1_p[:])
            nc.vector.tensor_copy(out=sfb[D:2 * D, :], in_=s2_p[:])
            mpb = cst.tile([72, 72], BF16)
            nc.scalar.copy(out=mpb[:], in_=mp_p[:])
            pb = cst.tile([128, S], BF16)
            nc.vector.tensor_copy(out=pb[:], in_=p_p[:])

            # OUT psum [96, 4S]: WU chunks at 0:3S, R at 3S:4S
            o_p = ps.tile([96, 4 * S], F32, tag="o")
            for uc in range(3):
                nc.tensor.matmul(out=o_p[0:96, uc * S:(uc + 1) * S],
                                 lhsT=sfb[:, 96 * uc:96 * (uc + 1)],
                                 rhs=pb[:], start=True, stop=True)
            nc.tensor.matmul(out=o_p[0:72, 3 * S:4 * S],
                             lhsT=sfb[:, UPAD:UPAD + 72], rhs=pb[:],
                             start=True, stop=False)
            nc.tensor.matmul(out=o_p[0:72, 3 * S:4 * S],
                             lhsT=mpb[:], rhs=oh, start=False, stop=True)

            # staged output: first half (WU chunks 0,1) ships while the
            # second half (chunk 2 + R) finishes
            nc.scalar.copy(out=sbo[:, 0:2 * S], in_=o_p[0:96, 0:2 * S])
            nc.sync.dma_start(out=out_d[:, 0:2 * S], in_=sbo[:, 0:2 * S])
            nc.scalar.copy(out=sbo[:, 2 * S:3 * S], in_=o_p[0:96, 2 * S:3 * S])
            nc.scalar.copy(out=sbo[0:72, 3 * S:4 * S],
                           in_=o_p[0:72, 3 * S:4 * S])
            nc.gpsimd.dma_start(out=out_d[:, 2 * S:4 * S],
                                in_=sbo[:, 2 * S:4 * S])

    _split_waits(nc)
    return nc


def _prepare(inputs):
    import ml_dtypes
    ins = {k: np.asarray(v) for k, v in inputs.items()}
    F1 = ins["F1"].astype(np.float32)
    F2 = ins["F2"].astype(np.float32)
    U1 = ins["U1"].astype(np.float32)
    U2 = ins["U2"].astype(np.float32)
    l1 = ins["lamda1"].astype(np.float32)
    l2 = ins["lamda2"].astype(np.float32)
    src1 = ins["src1"].astype(np.int64)
    dst1 = ins["dst1"].astype(np.int64)
    src2 = ins["src2"].astype(np.int64)
    dst2 = ins["dst2"].astype(np.int64)

    cols = np.arange(E)
    H1 = np.zeros((N, E), np.float32)
    H1[src1, cols] = 1.0
    H1[dst1, cols] = 1.0
    H2 = np.zeros((N, E), np.float32)
    H2[src2, cols] = 1.0
    H2[dst2, cols] = 1.0
    S1 = np.zeros((N, E), np.float32); S1[src1, cols] = 1.0
    D1m = np.zeros((N, E), np.float32); D1m[dst1, cols] = 1.0
    S2 = np.zeros((N, E), np.float32); S2[src2, cols] = 1.0
    D2m = np.zeros((N, E), np.float32); D2m[dst2, cols] = 1.0

    # unique graph-1 pairs (p<q), multi-edges merged; self-loops excluded
    pair_map = {}
    for i in range(E):
        p, q = int(src1[i]), int(dst1[i])
        if p == q:
            continue
        pair_map.setdefault((min(p, q), max(p, q)), []).append(i)
    plist1 = sorted(pair_map)
    G1 = np.zeros((E, UPAD), np.float32)
    for u, key in enumerate(plist1):
        for i in pair_map[key]:
            G1[i, u] = 1.0
    S2U = S2 @ G1
    D2U = D2m @ G1
    S2H = S2 @ H1.T
    D2H = D2m @ H1.T

    # slots: 9 diag per core + unique graph-2 pairs round-robin
    pairs2 = set()
    for j in range(E):
        a, c = int(src2[j]), int(dst2[j])
        if a != c:
            pairs2.add((min(a, c), max(a, c)))
    plist2 = sorted(pairs2)
    core_slots = [[(a, a) for a in range(9 * c, 9 * c + 9)] for c in range(NC)]
    for k, pr in enumerate(plist2):
        core_slots[k % NC].append(pr)
    assert all(len(s) <= S for s in core_slots)

    bf = ml_dtypes.bfloat16

    def put(dst, tab, nm, arr):
        o, r, w = tab[nm]
        dst[0:arr.shape[0], o:o + arr.shape[1]] = arr.astype(bf)

    PACKB1 = np.zeros((72, PB1_W), bf)
    put(PACKB1, PB1, "f2", F2)
    put(PACKB1, PB1, "s2u", S2U)
    put(PACKB1, PB1, "d2u", D2U)
    base_b2 = np.zeros((72, PB2_W), bf)
    put(base_b2, PB2, "s2h", S2H)
    put(base_b2, PB2, "d2h", D2H)
    put(base_b2, PB2, "u1t", np.ascontiguousarray(U1.T))
    put(base_b2, PB2, "u2t", np.ascontiguousarray(U2.T))
    base_a = np.zeros((72, PA_W), bf)
    put(base_a, PA, "l1", l1)
    put(base_a, PA, "l2", l2)
    put(base_a, PA, "f1t", np.ascontiguousarray(F1.T))

    SD1 = S1 + D1m
    in_maps = []
    for c in range(NC):
        slots = core_slots[c]
        SEL = np.zeros((E, S), np.float32)
        OH = np.zeros((N, S), np.float32)
        for s, (a, cc) in enumerate(slots):
            SEL[:, s] = H2[a] * H2[cc]
            if a == cc:
                OH[a, s] = 1.0
        pa = base_a.copy()
        put(pa, PA, "csd", SD1 @ SEL)
        pb2 = base_b2.copy()
        put(pb2, PB2, "oh", OH)
        in_maps.append({"PACKA": pa, "PACKB1": PACKB1, "PACKB2": pb2})
    aux = {"plist1": plist1, "core_slots": core_slots}
    return in_maps, aux


_CACHE = {}


def _get_nc():
    nc = _CACHE.get("nc")
    if nc is None:
        nc = _build_nc()
        _CACHE["nc"] = nc
    return nc


def kernel(**inputs):
    from concourse.bass_utils import run_bass_kernel_spmd

    in_maps, aux = _prepare(inputs)
    nc = _get_nc()
    res = run_bass_kernel_spmd(nc, in_maps, list(range(NC)))

    plist1 = aux["plist1"]
    nu = len(plist1)
    pu = np.array([p for p, q in plist1], np.int64)
    qu = np.array([q for p, q in plist1], np.int64)
    t = np.arange(N)
    M = np.zeros((N * N, N * N), np.float32)
    for c in range(NC):
        out = res.results[c]["OUT"]
        WU = np.concatenate([out[0:96, 0:S], out[0:96, S:2 * S],
                             out[0:96, 2 * S:3 * S]], axis=0)  # [288, S]
        R = out[0:N, 3 * S:4 * S]                               # [72, S]
        for s, (a, cc) in enumerate(aux["core_slots"][c]):
            v = WU[0:nu, s]
            M[a * N + pu, cc * N + qu] = v
            M[a * N + qu, cc * N + pu] = v
            M[a * N + t, cc * N + t] = R[:, s]
            if a != cc:
                M[cc * N + pu, a * N + qu] = v
                M[cc * N + qu, a * N + pu] = v
                M[cc * N + t, a * N + t] = R[:, s]
    return M


# revision 13
# speedup vs baseline: 1.1170x; 1.1170x over previous
"""Trainium2 Bass kernel for nn_Affinity (gnn_message_passing).

M[(a,b),(c,d)] = sum_{j,i} H2[a,j]H2[c,j] H1[b,i]H1[d,i] W[j,i] + diag(Mp),
W[j,i] = X[j] . lamda . Y[i]  (the reference's quirky Me reindexing).

Key structure: block B_{a,c}[b,d] = sum_i H1[b,i]H1[d,i] w_{ac}[i] is itself
sparse -- off-diagonal entries are (multi-edge-merged) values of w_{ac}, and
its diagonal is H1 @ w_{ac}. Folding the 0/1 selection tables (all host-built,
integer-valued) through the algebra, the whole per-core computation reduces to
~12 small matmuls producing two dense value tables:

  WU[u, s] = merged off-diag value of unique graph-1 pair u in slot s's block
  R[b, s]  = diagonal of slot s's block (+ Mp for diagonal slots)

with slots s = the ~360 unique (a,c) block pairs (72 diagonal + unique graph-2
edges), sharded 45-per-core over 8 cores. The host assembles the final
[5184, 5184] matrix by pure index scatter (no host float arithmetic).
"""
import sys
sys.path.insert(0, '/opt/trn_rl_repo')
import numpy as np

N = 72
E = 288
D = 64
NC = 8
S = 48          # slots per core (9 diag + <=36 edge pairs, padded)
UPAD = 288      # unique graph-1 pairs, padded

# PACKA layout [72, 256] bf16 (l1|l2 contiguous for a single relu)
PA = {"l1": (0, 64, 64), "l2": (64, 64, 64), "f1t": (128, 64, 72),
      "csd": (200, 72, S)}
PA_W = 256
# PACKB1 layout [72, 640] bf16
PB1 = {"f2": (0, 72, 64), "s2u": (64, 72, UPAD), "d2u": (352, 72, UPAD)}
PB1_W = 640
# PACKB2 layout [72, 336] bf16
PB2 = {"s2h": (0, 72, 72), "d2h": (72, 72, 72), "u1t": (144, 64, 72),
       "u2t": (216, 64, 72), "oh": (288, 72, S)}
PB2_W = 336


def _split_waits(nc, limit=1):
    """This walrus build rejects instructions with >limit sem waits; move the
    excess onto same-engine NoOps inserted immediately before."""
    import concourse.mybir as mybir
    for f in nc.m.functions:
        for bb in f.blocks:
            new_insts = []
            for inst in bb.instructions:
                si = inst.sync_info
                waits = list(si.on_wait) if si and si.on_wait else []
                if len(waits) > limit:
                    extra, keep = waits[:-limit], waits[-limit:]
                    for i in range(0, len(extra), limit):
                        nop = mybir.InstNoOp(
                            name=nc.get_next_instruction_name(),
                            engine=inst.engine, ins=[], outs=[],
                            sync_info=mybir.SyncInfo(
                                on_wait=extra[i:i + limit], on_update=[]),
                        )
                        nc.register_instruction(nop)
                        new_insts.append(nop)
                    si.on_wait = keep
                new_insts.append(inst)
            bb.instructions[:] = new_insts


def _build_nc():
    import concourse.bass as bass
    import concourse.mybir as mybir
    import concourse.tile as tile

    F32 = mybir.dt.float32
    BF16 = mybir.dt.bfloat16

    nc = bass.Bass(enable_partition_id=False)
    pka_d = nc.declare_dram_parameter("PACKA", [72, PA_W], BF16, isOutput=False)
    pkb1_d = nc.declare_dram_parameter("PACKB1", [72, PB1_W], BF16,
                                       isOutput=False)
    pkb2_d = nc.declare_dram_parameter("PACKB2", [72, PB2_W], BF16,
                                       isOutput=False)
    out_d = nc.declare_dram_parameter("OUT", [96, 4 * S], F32, isOutput=True)

    with tile.TileContext(nc) as tc:
        with tc.tile_pool(name="cst", bufs=1) as cst, \
             tc.tile_pool(name="ps", bufs=1, space="PSUM") as ps:

            pka = cst.tile([72, PA_W], BF16)
            nc.sync.dma_start(out=pka[:], in_=pka_d[:])
            pkb1 = cst.tile([72, PB1_W], BF16)
            nc.gpsimd.dma_start(out=pkb1[:], in_=pkb1_d[:])
            pkb2 = cst.tile([72, PB2_W], BF16)
            nc.scalar.dma_start(out=pkb2[:], in_=pkb2_d[:])

            def fld(pk, tab, nm):
                o, r, w = tab[nm]
                return pk[0:r, o:o + w]

            f1t = fld(pka, PA, "f1t")
            csd = fld(pka, PA, "csd")
            f2 = fld(pkb1, PB1, "f2")
            s2u = fld(pkb1, PB1, "s2u")
            d2u = fld(pkb1, PB1, "d2u")
            s2h = fld(pkb2, PB2, "s2h")
            d2h = fld(pkb2, PB2, "d2h")
            u1t = fld(pkb2, PB2, "u1t")
            u2t = fld(pkb2, PB2, "u2t")
            oh = fld(pkb2, PB2, "oh")

            # output staging; memset early so the padding rows of the R
            # region hold zeros rather than uninitialized SBUF
            sbo = cst.tile([96, 4 * S], F32)
            nc.vector.memset(sbo[:], 0.0)

            # relu of [lamda1 | lamda2] in one shot
            r12 = cst.tile([D, 2 * D], BF16)
            nc.vector.tensor_relu(out=r12[:], in_=pka[0:D, 0:2 * D])

            # FR = F1 @ [r1 | r2]  [72, 128]
            fr_p = ps.tile([72, 2 * D], F32, tag="fr")
            nc.tensor.matmul(out=fr_p[:], lhsT=f1t, rhs=r12[:],
                             start=True, stop=True)
            frb = cst.tile([72, 2 * D], BF16)
            nc.vector.tensor_copy(out=frb[:], in_=fr_p[:])

            # stage-1 feature/table products (independent of the FR chain)
            # s1 = [FSU(288) | FSR(72)],  s2 = [FDU(288) | FDR(72)]  [64, 360]
            s1_p = ps.tile([D, UPAD + 72], F32, tag="s1")
            nc.tensor.matmul(out=s1_p[:, 0:UPAD], lhsT=f2, rhs=s2u,
                             start=True, stop=True)
            nc.tensor.matmul(out=s1_p[:, UPAD:UPAD + 72], lhsT=f2, rhs=s2h,
                             start=True, stop=True)
            s2_p = ps.tile([D, UPAD + 72], F32, tag="s2")
            nc.tensor.matmul(out=s2_p[:, 0:UPAD], lhsT=f2, rhs=d2u,
                             start=True, stop=True)
            nc.tensor.matmul(out=s2_p[:, UPAD:UPAD + 72], lhsT=f2, rhs=d2h,
                             start=True, stop=True)
            mp_p = ps.tile([72, 72], F32, tag="mp")
            nc.tensor.matmul(out=mp_p[:], lhsT=u1t, rhs=u2t,
                             start=True, stop=True)

            # P = [P1 ; P2] = frb^T @ CSD  [128, S]
            p_p = ps.tile([128, S], F32, tag="p")
            nc.tensor.matmul(out=p_p[:], lhsT=frb[:], rhs=csd,
                             start=True, stop=True)

            # stacked [FSU;FDU | FSR;FDR] in SBUF: rows 0:64 from s1,
            # 64:128 from s2 (disjoint partitions, two engines in parallel)
            sfb = cst.tile([128, UPAD + 72], BF16)
            nc.scalar.copy(out=sfb[0:D, :], in_=s1_p[:])
            nc.vector.tensor_copy(out=sfb[D:2 * D, :], in_=s2_p[:])
            mpb = cst.tile([72, 72], BF16)
            nc.scalar.copy(out=mpb[:], in_=mp_p[:])
            pb = cst.tile([128, S], BF16)
            nc.vector.tensor_copy(out=pb[:], in_=p_p[:])

            # OUT psum [96, 4S]: WU chunks at 0:3S, R at 3S:4S
            o_p = ps.tile([96, 4 * S], F32, tag="o")
            for uc in range(3):
                nc.tensor.matmul(out=o_p[0:96, uc * S:(uc + 1) * S],
                                 lhsT=sfb[:, 96 * uc:96 * (uc + 1)],
                                 rhs=pb[:], start=True, stop=True)
            nc.tensor.matmul(out=o_p[0:72, 3 * S:4 * S],
                             lhsT=sfb[:, UPAD:UPAD + 72], rhs=pb[:],
                             start=True, stop=False)
            nc.tensor.matmul(out=o_p[0:72, 3 * S:4 * S],
                             lhsT=mpb[:], rhs=oh, start=False, stop=True)

            # staged output: first half (WU chunks 0,1) ships while the
            # second half (chunk 2 + R) finishes
            nc.scalar.copy(out=sbo[:, 0:2 * S], in_=o_p[0:96, 0:2 * S])
            nc.sync.dma_start(out=out_d[:, 0:2 * S], in_=sbo[:, 0:2 * S])
            nc.scalar.copy(out=sbo[:, 2 * S:3 * S], in_=o_p[0:96, 2 * S:3 * S])
            nc.scalar.copy(out=sbo[0:72, 3 * S:4 * S],
                           in_=o_p[0:72, 3 * S:4 * S])
            nc.gpsimd.dma_start(out=out_d[:, 2 * S:4 * S],
                                in_=sbo[:, 2 * S:4 * S])

    _split_waits(nc)
    return nc


def _build_nc_raw():
    """Raw Bass (no Tile scheduler): manual semaphores, hand-tuned engine
    assignment and ordering. Output DMAs read PSUM directly (no staging)."""
    from contextlib import ExitStack
    import concourse.bass as bass
    import concourse.mybir as mybir

    F32 = mybir.dt.float32
    BF16 = mybir.dt.bfloat16

    nc = bass.Bass(enable_partition_id=False)
    pka_d = nc.declare_dram_parameter("PACKA", [72, PA_W], BF16, isOutput=False)
    pkb1_d = nc.declare_dram_parameter("PACKB1", [72, PB1_W], BF16,
                                       isOutput=False)
    pkb2_d = nc.declare_dram_parameter("PACKB2", [72, PB2_W], BF16,
                                       isOutput=False)
    out_d = nc.declare_dram_parameter("OUT", [96, 4 * S], F32, isOutput=True)

    with ExitStack() as es:
        pka = es.enter_context(nc.sbuf_tensor([72, PA_W], BF16))
        pkb1 = es.enter_context(nc.sbuf_tensor([72, PB1_W], BF16))
        pkb2 = es.enter_context(nc.sbuf_tensor([72, PB2_W], BF16))
        r12 = es.enter_context(nc.sbuf_tensor([D, 2 * D], BF16))
        frb = es.enter_context(nc.sbuf_tensor([72, 2 * D], BF16))
        sfb = es.enter_context(nc.sbuf_tensor([128, UPAD + 72], BF16))
        mpb = es.enter_context(nc.sbuf_tensor([72, 72], BF16))
        pb = es.enter_context(nc.sbuf_tensor([128, S], BF16))
        fr_p = es.enter_context(nc.psum_tensor([72, 2 * D], F32))
        s1_p = es.enter_context(nc.psum_tensor([D, UPAD + 72], F32))
        s2_p = es.enter_context(nc.psum_tensor([D, UPAD + 72], F32))
        mp_p = es.enter_context(nc.psum_tensor([72, 72], F32))
        p_p = es.enter_context(nc.psum_tensor([128, S], F32))
        o_p = es.enter_context(nc.psum_tensor([96, 4 * S], F32))
        qA = es.enter_context(nc.semaphore("qA"))
        qB1 = es.enter_context(nc.semaphore("qB1"))
        qB2 = es.enter_context(nc.semaphore("qB2"))
        sV = es.enter_context(nc.semaphore("sV"))
        sT = es.enter_context(nc.semaphore("sT"))
        sS = es.enter_context(nc.semaphore("sS"))
        qO = es.enter_context(nc.semaphore("qO"))

        f1t = pka[0:D, 128:200]
        csd = pka[0:72, 200:200 + S]
        f2 = pkb1[0:72, 0:64]
        s2u = pkb1[0:72, 64:352]
        d2u = pkb1[0:72, 352:640]
        s2h = pkb2[0:72, 0:72]
        d2h = pkb2[0:72, 72:144]
        u1t = pkb2[0:D, 144:216]
        u2t = pkb2[0:D, 216:288]
        oh = pkb2[0:72, 288:288 + S]

        def mm(out, lhsT, rhs, start=True, stop=True):
            return nc.tensor.matmul(out=out, lhsT=lhsT, rhs=rhs,
                                    start=start, stop=stop,
                                    skip_group_check=True).then_inc(sT, 1)

        sbo = es.enter_context(nc.sbuf_tensor([96, 4 * S], F32))

        # --- sync (SP): inputs A and B2, then output DMAs from staged SBUF ---
        nc.sync.dma_start(out=pka[:], in_=pka_d[:]).then_inc(qA, 16)
        nc.sync.dma_start(out=pkb2[:], in_=pkb2_d[:]).then_inc(qB2, 16)
        nc.sync.wait_ge(sS, 3)
        nc.sync.dma_start(out=out_d[:, 0:2 * S],
                          in_=sbo[:, 0:2 * S]).then_inc(qO, 16)
        nc.sync.wait_ge(sS, 5)
        nc.sync.dma_start(out=out_d[:, 2 * S:4 * S],
                          in_=sbo[:, 2 * S:4 * S]).then_inc(qO, 16)

        # --- tensor (PE): all matmuls ---
        nc.tensor.wait_ge(qA, 16)
        nc.tensor.wait_ge(sV, 1)
        mm(fr_p[:], f1t, r12[:])                                   # 1 FR
        nc.tensor.wait_ge(qB1, 16)
        mm(s1_p[0:D, 0:UPAD], f2, s2u)                             # 2 FSU
        mm(s2_p[0:D, 0:UPAD], f2, d2u)                             # 3 FDU
        nc.tensor.wait_ge(qB2, 16)
        mm(s1_p[0:D, UPAD:UPAD + 72], f2, s2h)                     # 4 FSR
        mm(s2_p[0:D, UPAD:UPAD + 72], f2, d2h)                     # 5 FDR
        mm(mp_p[:], u1t, u2t)                                      # 6 MP
        nc.tensor.wait_ge(sV, 2)
        mm(p_p[:], frb[:], csd)                                    # 7 P
        nc.tensor.wait_ge(sV, 4)
        nc.tensor.wait_ge(sS, 1)
        for uc in range(3):                                        # 8,9,10 WU
            mm(o_p[0:96, uc * S:(uc + 1) * S],
               sfb[:, 96 * uc:96 * (uc + 1)], pb[:])
        mm(o_p[0:72, 3 * S:4 * S], sfb[:, UPAD:UPAD + 72], pb[:],
           start=True, stop=False)                                 # 11 R1
        nc.tensor.wait_ge(sS, 2)
        mm(o_p[0:72, 3 * S:4 * S], mpb[:], oh,
           start=False, stop=True)                                 # 12 R2

        # --- vector (DVE): relu + critical casts ---
        nc.vector.wait_ge(qA, 16)
        nc.vector.tensor_relu(out=r12[:], in_=pka[0:D, 0:2 * D]).then_inc(sV, 1)
        nc.vector.wait_ge(sT, 1)
        nc.vector.tensor_copy(out=frb[:], in_=fr_p[:]).then_inc(sV, 1)
        nc.vector.wait_ge(sT, 5)
        nc.vector.tensor_copy(out=sfb[D:2 * D, :], in_=s2_p[:]).then_inc(sV, 1)
        nc.vector.wait_ge(sT, 7)
        nc.vector.tensor_copy(out=pb[:], in_=p_p[:]).then_inc(sV, 1)

        # --- scalar (ACT): input B1 + off-critical casts + output staging ---
        nc.scalar.dma_start(out=pkb1[:], in_=pkb1_d[:]).then_inc(qB1, 16)
        nc.scalar.wait_ge(sT, 4)
        nc.scalar.copy(out=sfb[0:D, :], in_=s1_p[:]).then_inc(sS, 1)
        nc.scalar.wait_ge(sT, 6)
        nc.scalar.copy(out=mpb[:], in_=mp_p[:]).then_inc(sS, 1)
        nc.scalar.wait_ge(sT, 9)
        nc.scalar.copy(out=sbo[:, 0:2 * S],
                       in_=o_p[0:96, 0:2 * S]).then_inc(sS, 1)
        nc.scalar.wait_ge(sT, 10)
        nc.scalar.copy(out=sbo[:, 2 * S:3 * S],
                       in_=o_p[0:96, 2 * S:3 * S]).then_inc(sS, 1)
        nc.scalar.wait_ge(sT, 12)
        nc.scalar.copy(out=sbo[0:72, 3 * S:4 * S],
                       in_=o_p[0:72, 3 * S:4 * S]).then_inc(sS, 1)

    _split_waits(nc)
    return nc


def _prepare(inputs):
    import ml_dtypes
    ins = {k: np.asarray(v) for k, v in inputs.items()}
    F1 = ins["F1"].astype(np.float32)
    F2 = ins["F2"].astype(np.float32)
    U1 = ins["U1"].astype(np.float32)
    U2 = ins["U2"].astype(np.float32)
    l1 = ins["lamda1"].astype(np.float32)
    l2 = ins["lamda2"].astype(np.float32)
    src1 = ins["src1"].astype(np.int64)
    dst1 = ins["dst1"].astype(np.int64)
    src2 = ins["src2"].astype(np.int64)
    dst2 = ins["dst2"].astype(np.int64)

    cols = np.arange(E)
    H1 = np.zeros((N, E), np.float32)
    H1[src1, cols] = 1.0
    H1[dst1, cols] = 1.0
    H2 = np.zeros((N, E), np.float32)
    H2[src2, cols] = 1.0
    H2[dst2, cols] = 1.0
    S1 = np.zeros((N, E), np.float32); S1[src1, cols] = 1.0
    D1m = np.zeros((N, E), np.float32); D1m[dst1, cols] = 1.0
    S2 = np.zeros((N, E), np.float32); S2[src2, cols] = 1.0
    D2m = np.zeros((N, E), np.float32); D2m[dst2, cols] = 1.0

    # unique graph-1 pairs (p<q), multi-edges merged; self-loops excluded
    pair_map = {}
    for i in range(E):
        p, q = int(src1[i]), int(dst1[i])
        if p == q:
            continue
        pair_map.setdefault((min(p, q), max(p, q)), []).append(i)
    plist1 = sorted(pair_map)
    G1 = np.zeros((E, UPAD), np.float32)
    for u, key in enumerate(plist1):
        for i in pair_map[key]:
            G1[i, u] = 1.0
    S2U = S2 @ G1
    D2U = D2m @ G1
    S2H = S2 @ H1.T
    D2H = D2m @ H1.T

    # slots: 9 diag per core + unique graph-2 pairs round-robin
    pairs2 = set()
    for j in range(E):
        a, c = int(src2[j]), int(dst2[j])
        if a != c:
            pairs2.add((min(a, c), max(a, c)))
    plist2 = sorted(pairs2)
    core_slots = [[(a, a) for a in range(9 * c, 9 * c + 9)] for c in range(NC)]
    for k, pr in enumerate(plist2):
        core_slots[k % NC].append(pr)
    assert all(len(s) <= S for s in core_slots)

    bf = ml_dtypes.bfloat16

    def put(dst, tab, nm, arr):
        o, r, w = tab[nm]
        dst[0:arr.shape[0], o:o + arr.shape[1]] = arr.astype(bf)

    PACKB1 = np.zeros((72, PB1_W), bf)
    put(PACKB1, PB1, "f2", F2)
    put(PACKB1, PB1, "s2u", S2U)
    put(PACKB1, PB1, "d2u", D2U)
    base_b2 = np.zeros((72, PB2_W), bf)
    put(base_b2, PB2, "s2h", S2H)
    put(base_b2, PB2, "d2h", D2H)
    put(base_b2, PB2, "u1t", np.ascontiguousarray(U1.T))
    put(base_b2, PB2, "u2t", np.ascontiguousarray(U2.T))
    base_a = np.zeros((72, PA_W), bf)
    put(base_a, PA, "l1", l1)
    put(base_a, PA, "l2", l2)
    put(base_a, PA, "f1t", np.ascontiguousarray(F1.T))

    SD1 = S1 + D1m
    in_maps = []
    for c in range(NC):
        slots = core_slots[c]
        SEL = np.zeros((E, S), np.float32)
        OH = np.zeros((N, S), np.float32)
        for s, (a, cc) in enumerate(slots):
            SEL[:, s] = H2[a] * H2[cc]
            if a == cc:
                OH[a, s] = 1.0
        pa = base_a.copy()
        put(pa, PA, "csd", SD1 @ SEL)
        pb2 = base_b2.copy()
        put(pb2, PB2, "oh", OH)
        in_maps.append({"PACKA": pa, "PACKB1": PACKB1, "PACKB2": pb2})
    aux = {"plist1": plist1, "core_slots": core_slots}
    return in_maps, aux


_CACHE = {}


def _get_nc():
    nc = _CACHE.get("nc")
    if nc is None:
        nc = _build_nc_raw()
        _CACHE["nc"] = nc
    return nc


def kernel(**inputs):
    from concourse.bass_utils import run_bass_kernel_spmd

    in_maps, aux = _prepare(inputs)
    nc = _get_nc()
    res = run_bass_kernel_spmd(nc, in_maps, list(range(NC)))

    plist1 = aux["plist1"]
    nu = len(plist1)
    pu = np.array([p for p, q in plist1], np.int64)
    qu = np.array([q for p, q in plist1], np.int64)
    t = np.arange(N)
    M = np.zeros((N * N, N * N), np.float32)
    for c in range(NC):
        out = res.results[c]["OUT"]
        WU = np.concatenate([out[0:96, 0:S], out[0:96, S:2 * S],
                             out[0:96, 2 * S:3 * S]], axis=0)  # [288, S]
        R = out[0:N, 3 * S:4 * S]                               # [72, S]
        for s, (a, cc) in enumerate(aux["core_slots"][c]):
            v = WU[0:nu, s]
            M[a * N + pu, cc * N + qu] = v
            M[a * N + qu, cc * N + pu] = v
            M[a * N + t, cc * N + t] = R[:, s]
            if a != cc:
                M[cc * N + pu, a * N + qu] = v
                M[cc * N + qu, a * N + pu] = v
                M[cc * N + t, a * N + t] = R[:, s]
    return M


# revision 25
# speedup vs baseline: 1.2042x; 1.0780x over previous
"""Trainium2 Bass kernel for nn_Affinity (gnn_message_passing).

M[(a,b),(c,d)] = sum_{j,i} H2[a,j]H2[c,j] H1[b,i]H1[d,i] W[j,i] + diag(Mp),
W[j,i] = X[j] . lamda . Y[i]  (the reference's quirky Me reindexing).

Key structure: block B_{a,c}[b,d] = sum_i H1[b,i]H1[d,i] w_{ac}[i] is itself
sparse -- off-diagonal entries are (multi-edge-merged) values of w_{ac}, and
its diagonal is H1 @ w_{ac}. Folding the 0/1 selection tables (all host-built,
integer-valued) through the algebra, the whole per-core computation reduces to
12 small matmuls producing two dense value tables:

  WU[u, s] = merged off-diag value of unique graph-1 pair u in slot s's block
  R[b, s]  = diagonal of slot s's block (+ Mp for diagonal slots)

with slots s = the ~360 unique (a,c) block pairs (72 diagonal + unique graph-2
edges), sharded 45-per-core over 8 cores. The host assembles the final
[5184, 5184] matrix by pure index scatter (no host float arithmetic).

Raw Bass (no Tile scheduler): manual semaphores, hand-tuned engine assignment
and instruction order.
"""
import sys
sys.path.insert(0, '/opt/trn_rl_repo')
import numpy as np

N = 72
E = 288
D = 64
NC = 8
S = 48          # slots per core (9 diag + <=36 edge pairs, padded)
UPAD = 288      # unique graph-1 pairs, padded

# PACKA layout [72, 256] bf16 (l1|l2 contiguous for a single relu)
PA = {"l1": (0, 64, 64), "l2": (64, 64, 64), "f1t": (128, 64, 72),
      "csd": (200, 72, S)}
PA_W = 256
# PACKB1 layout [72, 640] bf16
PB1 = {"f2": (0, 72, 64), "s2u": (64, 72, UPAD), "d2u": (352, 72, UPAD)}
PB1_W = 640
# PACKB2 layout [72, 336] bf16
PB2 = {"s2h": (0, 72, 72), "d2h": (72, 72, 72), "u1t": (144, 64, 72),
       "u2t": (216, 64, 72), "oh": (288, 72, S)}
PB2_W = 336


def _split_waits(nc, limit=1):
    """This walrus build rejects instructions with >limit sem waits; move the
    excess onto same-engine NoOps inserted immediately before."""
    import concourse.mybir as mybir
    for f in nc.m.functions:
        for bb in f.blocks:
            new_insts = []
            for inst in bb.instructions:
                si = inst.sync_info
                waits = list(si.on_wait) if si and si.on_wait else []
                if len(waits) > limit:
                    extra, keep = waits[:-limit], waits[-limit:]
                    for i in range(0, len(extra), limit):
                        nop = mybir.InstNoOp(
                            name=nc.get_next_instruction_name(),
                            engine=inst.engine, ins=[], outs=[],
                            sync_info=mybir.SyncInfo(
                                on_wait=extra[i:i + limit], on_update=[]),
                        )
                        nc.register_instruction(nop)
                        new_insts.append(nop)
                    si.on_wait = keep
                new_insts.append(inst)
            bb.instructions[:] = new_insts


def _build_nc_raw():
    from contextlib import ExitStack
    import concourse.bass as bass
    import concourse.mybir as mybir

    F32 = mybir.dt.float32
    BF16 = mybir.dt.bfloat16

    nc = bass.Bass(enable_partition_id=False)
    pka_d = nc.declare_dram_parameter("PACKA", [72, PA_W], BF16, isOutput=False)
    pkb1_d = nc.declare_dram_parameter("PACKB1", [72, PB1_W], BF16,
                                       isOutput=False)
    pkb2_d = nc.declare_dram_parameter("PACKB2", [72, PB2_W], BF16,
                                       isOutput=False)
    out_d = nc.declare_dram_parameter("OUT", [96, 4 * S], F32, isOutput=True)

    with ExitStack() as es:
        pka = es.enter_context(nc.sbuf_tensor([72, PA_W], BF16))
        pkb1 = es.enter_context(nc.sbuf_tensor([72, PB1_W], BF16))
        pkb2 = es.enter_context(nc.sbuf_tensor([72, PB2_W], BF16))
        r12 = es.enter_context(nc.sbuf_tensor([D, 2 * D], BF16))
        frb = es.enter_context(nc.sbuf_tensor([72, 2 * D], BF16))
        sfb = es.enter_context(nc.sbuf_tensor([128, UPAD + 72], BF16))
        mpb = es.enter_context(nc.sbuf_tensor([72, 72], BF16))
        pb = es.enter_context(nc.sbuf_tensor([128, S], BF16))
        sbo = es.enter_context(nc.sbuf_tensor([96, 4 * S], F32))
        scrap = es.enter_context(nc.sbuf_tensor([1, 2], BF16))
        fr_p = es.enter_context(nc.psum_tensor([72, 2 * D], F32))
        s1_p = es.enter_context(nc.psum_tensor([D, UPAD + 72], F32))
        s2_p = es.enter_context(nc.psum_tensor([D, UPAD + 72], F32))
        mp_p = es.enter_context(nc.psum_tensor([72, 72], F32))
        p_p = es.enter_context(nc.psum_tensor([128, S], F32))
        o_p = es.enter_context(nc.psum_tensor([96, 4 * S], F32))
        qA = es.enter_context(nc.semaphore("qA"))
        qB1 = es.enter_context(nc.semaphore("qB1"))
        qB2 = es.enter_context(nc.semaphore("qB2"))
        sV = es.enter_context(nc.semaphore("sV"))
        sT = es.enter_context(nc.semaphore("sT"))
        sS = es.enter_context(nc.semaphore("sS"))
        sG = es.enter_context(nc.semaphore("sG"))
        qO = es.enter_context(nc.semaphore("qO"))

        def fld(pk, tab, nm):
            o, r, w = tab[nm]
            return pk[0:r, o:o + w]

        f1t, csd = fld(pka, PA, "f1t"), fld(pka, PA, "csd")
        s2h, d2h = fld(pkb2, PB2, "s2h"), fld(pkb2, PB2, "d2h")
        u1t, u2t, oh = (fld(pkb2, PB2, "u1t"), fld(pkb2, PB2, "u2t"),
                        fld(pkb2, PB2, "oh"))
        f2 = fld(pkb1, PB1, "f2")
        s2u = fld(pkb1, PB1, "s2u")
        d2u = fld(pkb1, PB1, "d2u")

        def mm(out, lhsT, rhs, start=True, stop=True):
            return nc.tensor.matmul(out=out, lhsT=lhsT, rhs=rhs,
                                    start=start, stop=stop,
                                    skip_group_check=True).then_inc(sT, 1)

        # --- sync (SP): inputs A+B2, output DMAs from staged SBUF ---
        nc.sync.dma_start(out=pka[:], in_=pka_d[:]).then_inc(qA, 16)
        nc.sync.dma_start(out=pkb2[:], in_=pkb2_d[:]).then_inc(qB2, 16)
        nc.sync.wait_ge(sS, 3)
        nc.sync.dma_start(out=out_d[:, 0:2 * S],
                          in_=sbo[:, 0:2 * S]).then_inc(qO, 16)
        nc.sync.wait_ge(sS, 5)
        nc.sync.dma_start(out=out_d[:, 2 * S:4 * S],
                          in_=sbo[:, 2 * S:4 * S]).then_inc(qO, 16)


        # --- tensor (PE): all matmuls, order tuned to unblock casts early ---
        nc.tensor.wait_ge(qA, 16)
        nc.tensor.wait_ge(sV, 1)
        mm(fr_p[:], f1t, r12[:])                                   # 1 FR
        nc.tensor.wait_ge(qB1, 16)
        mm(s1_p[0:D, 0:UPAD], f2, s2u)                             # 2 FSU
        mm(s2_p[0:D, 0:UPAD], f2, d2u)                             # 3 FDU
        nc.tensor.wait_ge(qB2, 16)
        mm(s1_p[0:D, UPAD:UPAD + 72], f2, s2h)                     # 4 FSR
        mm(s2_p[0:D, UPAD:UPAD + 72], f2, d2h)                     # 5 FDR
        mm(mp_p[:], u1t, u2t)                                      # 6 MP
        nc.tensor.wait_ge(sV, 2)
        mm(p_p[:], frb[:], csd)                                    # 7 P
        nc.tensor.wait_ge(sV, 4)
        nc.tensor.wait_ge(sS, 1)
        for uc in range(3):                                        # 8,9,10 WU
            mm(o_p[0:96, uc * S:(uc + 1) * S],
               sfb[:, 96 * uc:96 * (uc + 1)], pb[:])
        mm(o_p[0:72, 3 * S:4 * S], sfb[:, UPAD:UPAD + 72], pb[:],
           start=True, stop=False)                                 # 11 R1
        nc.tensor.wait_ge(sS, 2)
        mm(o_p[0:72, 3 * S:4 * S], mpb[:], oh,
           start=False, stop=True)                                 # 12 R2

        # --- vector (DVE): relu + critical casts ---
        nc.vector.wait_ge(qA, 16)
        nc.vector.tensor_relu(out=r12[:], in_=pka[0:D, 0:2 * D]).then_inc(sV, 1)
        nc.vector.wait_ge(sT, 1)
        nc.vector.tensor_copy(out=frb[:], in_=fr_p[:]).then_inc(sV, 1)
        nc.vector.wait_ge(sT, 5)
        nc.vector.tensor_copy(out=sfb[D:2 * D, :], in_=s2_p[:]).then_inc(sV, 1)
        nc.vector.wait_ge(sT, 7)
        nc.vector.tensor_copy(out=pb[:], in_=p_p[:]).then_inc(sV, 1)

        # --- scalar (ACT): input B1 + casts + output staging ---
        nc.scalar.dma_start(out=pkb1[:], in_=pkb1_d[:]).then_inc(qB1, 16)
        nc.scalar.copy(out=scrap[0:1, 1:2], in_=scrap[0:1, 0:1])
        nc.scalar.wait_ge(sT, 4)
        nc.scalar.copy(out=sfb[0:D, :], in_=s1_p[:]).then_inc(sS, 1)
        nc.scalar.wait_ge(sT, 6)
        nc.scalar.copy(out=mpb[:], in_=mp_p[:]).then_inc(sS, 1)
        nc.scalar.wait_ge(sT, 9)
        nc.scalar.copy(out=sbo[:, 0:2 * S],
                       in_=o_p[0:96, 0:2 * S]).then_inc(sS, 1)
        nc.scalar.wait_ge(sT, 10)
        nc.scalar.copy(out=sbo[:, 2 * S:3 * S],
                       in_=o_p[0:96, 2 * S:3 * S]).then_inc(sS, 1)
        nc.scalar.wait_ge(sT, 12)
        nc.scalar.copy(out=sbo[0:72, 3 * S:4 * S],
                       in_=o_p[0:72, 3 * S:4 * S]).then_inc(sS, 1)

    _split_waits(nc)
    return nc


def _prepare(inputs):
    import ml_dtypes
    ins = {k: np.asarray(v) for k, v in inputs.items()}
    F1 = ins["F1"].astype(np.float32)
    F2 = ins["F2"].astype(np.float32)
    U1 = ins["U1"].astype(np.float32)
    U2 = ins["U2"].astype(np.float32)
    l1 = ins["lamda1"].astype(np.float32)
    l2 = ins["lamda2"].astype(np.float32)
    src1 = ins["src1"].astype(np.int64)
    dst1 = ins["dst1"].astype(np.int64)
    src2 = ins["src2"].astype(np.int64)
    dst2 = ins["dst2"].astype(np.int64)

    cols = np.arange(E)
    H1 = np.zeros((N, E), np.float32)
    H1[src1, cols] = 1.0
    H1[dst1, cols] = 1.0
    H2 = np.zeros((N, E), np.float32)
    H2[src2, cols] = 1.0
    H2[dst2, cols] = 1.0
    S1 = np.zeros((N, E), np.float32); S1[src1, cols] = 1.0
    D1m = np.zeros((N, E), np.float32); D1m[dst1, cols] = 1.0
    S2 = np.zeros((N, E), np.float32); S2[src2, cols] = 1.0
    D2m = np.zeros((N, E), np.float32); D2m[dst2, cols] = 1.0

    # unique graph-1 pairs (p<q), multi-edges merged; self-loops excluded
    pair_map = {}
    for i in range(E):
        p, q = int(src1[i]), int(dst1[i])
        if p == q:
            continue
        pair_map.setdefault((min(p, q), max(p, q)), []).append(i)
    plist1 = sorted(pair_map)
    G1 = np.zeros((E, UPAD), np.float32)
    for u, key in enumerate(plist1):
        for i in pair_map[key]:
            G1[i, u] = 1.0
    S2U = S2 @ G1
    D2U = D2m @ G1
    S2H = S2 @ H1.T
    D2H = D2m @ H1.T

    # slots: 9 diag per core + unique graph-2 pairs round-robin
    pairs2 = set()
    for j in range(E):
        a, c = int(src2[j]), int(dst2[j])
        if a != c:
            pairs2.add((min(a, c), max(a, c)))
    plist2 = sorted(pairs2)
    core_slots = [[(a, a) for a in range(9 * c, 9 * c + 9)] for c in range(NC)]
    for k, pr in enumerate(plist2):
        core_slots[k % NC].append(pr)
    assert all(len(s) <= S for s in core_slots)

    bf = ml_dtypes.bfloat16

    def put(dst, tab, nm, arr):
        o, r, w = tab[nm]
        dst[0:arr.shape[0], o:o + arr.shape[1]] = arr.astype(bf)

    PACKB1 = np.zeros((72, PB1_W), bf)
    put(PACKB1, PB1, "f2", F2)
    put(PACKB1, PB1, "s2u", S2U)
    put(PACKB1, PB1, "d2u", D2U)
    base_a = np.zeros((72, PA_W), bf)
    put(base_a, PA, "l1", l1)
    put(base_a, PA, "l2", l2)
    put(base_a, PA, "f1t", np.ascontiguousarray(F1.T))
    base_b2 = np.zeros((72, PB2_W), bf)
    put(base_b2, PB2, "s2h", S2H)
    put(base_b2, PB2, "d2h", D2H)
    put(base_b2, PB2, "u1t", np.ascontiguousarray(U1.T))
    put(base_b2, PB2, "u2t", np.ascontiguousarray(U2.T))

    SD1 = S1 + D1m
    in_maps = []
    for c in range(NC):
        slots = core_slots[c]
        SEL = np.zeros((E, S), np.float32)
        OH = np.zeros((N, S), np.float32)
        for s, (a, cc) in enumerate(slots):
            SEL[:, s] = H2[a] * H2[cc]
            if a == cc:
                OH[a, s] = 1.0
        pa = base_a.copy()
        put(pa, PA, "csd", SD1 @ SEL)
        pb2 = base_b2.copy()
        put(pb2, PB2, "oh", OH)
        in_maps.append({"PACKA": pa, "PACKB1": PACKB1, "PACKB2": pb2})
    aux = {"plist1": plist1, "core_slots": core_slots}
    return in_maps, aux


_CACHE = {}


def _get_nc():
    nc = _CACHE.get("nc")
    if nc is None:
        nc = _build_nc_raw()
        _CACHE["nc"] = nc
    return nc


def kernel(**inputs):
    from concourse.bass_utils import run_bass_kernel_spmd

    in_maps, aux = _prepare(inputs)
    nc = _get_nc()
    res = run_bass_kernel_spmd(nc, in_maps, list(range(NC)))

    plist1 = aux["plist1"]
    nu = len(plist1)
    pu = np.array([p for p, q in plist1], np.int64)
    qu = np.array([q for p, q in plist1], np.int64)
    t = np.arange(N)
    M = np.zeros((N * N, N * N), np.float32)
    for c in range(NC):
        out = res.results[c]["OUT"]
        WU = np.concatenate([out[0:96, 0:S], out[0:96, S:2 * S],
                             out[0:96, 2 * S:3 * S]], axis=0)  # [288, S]
        R = out[0:N, 3 * S:4 * S]                               # [72, S]
        for s, (a, cc) in enumerate(aux["core_slots"][c]):
            v = WU[0:nu, s]
            M[a * N + pu, cc * N + qu] = v
            M[a * N + qu, cc * N + pu] = v
            M[a * N + t, cc * N + t] = R[:, s]
            if a != cc:
                M[cc * N + pu, a * N + qu] = v
                M[cc * N + qu, a * N + pu] = v
                M[cc * N + t, a * N + t] = R[:, s]
    return M
